# revision 9
# baseline (speedup 1.0000x reference)
"""Jagged append kernel for Trainium2 (8 NeuronCores, SPMD).

out = concat_i( values[seg_i] ++ suffix_mat[i] )   for B jagged segments.

Strategy (per core, B/8 contiguous segments, embarrassingly parallel):
  - Segment offsets (prefix_sum) are host-visible, so all DMA destination
    offsets are precomputed on host and shipped as int32 index tables.
  - The core's values slice is staged DRAM->SBUF with large contiguous
    HWDGE loads (full bandwidth), then scattered SBUF->DRAM with indirect
    SWDGE DMAs: one descriptor per 256-element block, destination =
    block_start + 256*seg(block_start), element-granularity offsets
    (out viewed as [N, 1], offset axis 0 => coef 1). Each indirect DMA
    instruction scatters 128 blocks (one per SBUF partition; payload =
    one 256-wide column slice of a staging tile).
  - A grid block that spans a segment boundary writes its tail past the
    segment's values; that tail lands exactly inside the segment's
    256-element suffix region of the output (garbage, overwritten later).
    The next segment's leading values (up to 255 elements, until the grid
    realigns) are covered by a REPAIR pass: per segment r>=1 one block
    values[end_{r-1} : end_{r-1}+256) -> out[end_{r-1} + 256*r]. Repair
    blocks overlap neighbouring grid blocks only with identical data, so
    the repair pass needs no ordering. Repair data is pre-gathered on
    host and shipped as an input tile.
  - A SUFFIX pass then scatters the 256-element suffix rows, overwriting
    the garbage; it is ordered after the grid pass with DMA completion
    semaphores.
  - Pad/dummy blocks (to make all 8 cores shape-identical) go to a
    scratch tail of the output tensor that the host discards.
"""

import math
import os

import numpy as np

# filled by kernel() when BASS_KERNEL_TRACE=1: exec_time_ns, trace path
LAST_EXEC_NS = {}


def _np(x, dtype):
    return np.ascontiguousarray(np.asarray(x), dtype=dtype)


def _host_reference(values, prefix_sum, suffix_mat):
    """Numpy fallback matching reference() exactly (incl. its clamping),
    used only for off-distribution inputs the fast path can't handle."""
    b, suf = suffix_mat.shape
    n_vals = values.shape[0]
    n_out = n_vals + b * suf
    new_prefix = prefix_sum + np.arange(1, b + 1, dtype=prefix_sum.dtype) * suf
    j = np.arange(n_out, dtype=prefix_sum.dtype)
    seg = np.searchsorted(new_prefix, j, side="right")
    seg_c = np.clip(seg, 0, b - 1)
    prev_old = np.where(seg > 0, prefix_sum[np.clip(seg - 1, 0, b - 1)], 0)
    new_start = prev_old + seg * suf
    local = j - new_start
    seg_len = prefix_sum[seg_c] - prev_old
    in_vals = local < seg_len
    val_idx = np.clip(prev_old + local, 0, n_vals - 1)
    suf_idx = np.clip(local - seg_len, 0, suf - 1)
    return np.where(in_vals, values[val_idx], suffix_mat[seg_c, suf_idx]).astype(
        np.float32
    )


def _build_program(
    chunk_cols, n_idx_cols, suf_cols, n_suf_scat, out_len, suf_w, n_iters=1
):
    """Build the SPMD bass program (identical on all 8 cores).

    n_iters > 1 repeats the whole pipeline back-to-back inside one NEFF
    (for benchmarking): semaphore thresholds scale with the iteration, and
    each iteration's loads wait for the previous iteration's scatters
    (SBUF WAR), so no semaphore reset is needed.
    """
    import concourse.bass as bass
    from concourse import bacc, mybir

    f32 = mybir.dt.float32
    i32 = mybir.dt.int32

    nc = bacc.Bacc("TRN2", target_bir_lowering=False, debug=False, num_devices=8)

    vals_d = [
        nc.dram_tensor(f"vals{g}", [128, w], f32, kind="ExternalInput").ap()
        for g, w in enumerate(chunk_cols)
    ]
    idxv_d = nc.dram_tensor("idxv", [128, n_idx_cols], i32, kind="ExternalInput").ap()
    idxs_d = nc.dram_tensor("idxs", [128, n_suf_scat], i32, kind="ExternalInput").ap()
    idxr_d = nc.dram_tensor("idxr", [128, n_suf_scat], i32, kind="ExternalInput").ap()
    sufx_d = nc.dram_tensor("sufx", [128, suf_cols], f32, kind="ExternalInput").ap()
    repd_d = nc.dram_tensor("repd", [128, suf_cols], f32, kind="ExternalInput").ap()
    out_d = nc.dram_tensor("out", [out_len, 1], f32, kind="ExternalOutput").ap()

    data_t = [
        nc.alloc_sbuf_tensor(f"data{g}", [128, w], f32).ap()
        for g, w in enumerate(chunk_cols)
    ]
    idxv_t = nc.alloc_sbuf_tensor("idxv_t", [128, n_idx_cols], i32).ap()
    idxs_t = nc.alloc_sbuf_tensor("idxs_t", [128, n_suf_scat], i32).ap()
    idxr_t = nc.alloc_sbuf_tensor("idxr_t", [128, n_suf_scat], i32).ap()
    sufx_t = nc.alloc_sbuf_tensor("sufx_t", [128, suf_cols], f32).ap()
    repd_t = nc.alloc_sbuf_tensor("repd_t", [128, suf_cols], f32).ap()

    G = 256
    n_chunks = len(chunk_cols)
    n_v_scat = sum(w // G for w in chunk_cols)

    with nc.Block() as block:
        s_idx = nc.alloc_semaphore("s_idx")
        s_ld = [nc.alloc_semaphore(f"s_ld{g}") for g in range(n_chunks)]
        s_rld = nc.alloc_semaphore("s_rld")
        s_suf = nc.alloc_semaphore("s_suf")
        s_vs = nc.alloc_semaphore("s_vs")
        s_rs = nc.alloc_semaphore("s_rs")
        s_ss = nc.alloc_semaphore("s_ss")

        @block.sync
        def _(sync: bass.BassEngine):
            sync.dma_start(out=idxv_t[:], in_=idxv_d[:]).then_inc(s_idx, 16)
            sync.dma_start(out=idxs_t[:], in_=idxs_d[:]).then_inc(s_idx, 16)
            sync.dma_start(out=idxr_t[:], in_=idxr_d[:]).then_inc(s_idx, 16)
            for t in range(n_iters):
                if t > 0:
                    # SBUF WAR: previous iteration's scatters must be done
                    sync.wait_ge(s_vs, 16 * n_v_scat * t)
                    sync.wait_ge(s_rs, 16 * n_suf_scat * t)
                    sync.wait_ge(s_ss, 16 * n_suf_scat * t)
                for g in range(n_chunks):
                    sync.dma_start(out=data_t[g][:], in_=vals_d[g][:]).then_inc(
                        s_ld[g], 16
                    )
                sync.dma_start(out=repd_t[:], in_=repd_d[:]).then_inc(s_rld, 16)
                sync.dma_start(out=sufx_t[:], in_=sufx_d[:]).then_inc(s_suf, 16)

        @block.gpsimd
        def _(gpsimd: bass.BassEngine):
            gpsimd.wait_ge(s_idx, 48)
            for t in range(n_iters):
                col = 0
                for g, w in enumerate(chunk_cols):
                    gpsimd.wait_ge(s_ld[g], 16 * (t + 1))
                    for c in range(w // G):
                        gpsimd.indirect_dma_start(
                            out=out_d[:],
                            out_offset=bass.IndirectOffsetOnAxis(
                                ap=idxv_t[:, col : col + 1], axis=0
                            ),
                            in_=data_t[g][:, c * G : (c + 1) * G],
                            in_offset=None,
                        ).then_inc(s_vs, 16)
                        col += 1
                # repair pass: unordered wrt grid/suffix (identical-data overlaps)
                gpsimd.wait_ge(s_rld, 16 * (t + 1))
                for cc in range(n_suf_scat):
                    gpsimd.indirect_dma_start(
                        out=out_d[:],
                        out_offset=bass.IndirectOffsetOnAxis(
                            ap=idxr_t[:, cc : cc + 1], axis=0
                        ),
                        in_=repd_t[:, cc * G : (cc + 1) * G],
                        in_offset=None,
                    ).then_inc(s_rs, 16)
                # suffix pass strictly after every grid write has landed
                gpsimd.wait_ge(s_vs, 16 * n_v_scat * (t + 1))
                gpsimd.wait_ge(s_suf, 16 * (t + 1))
                for cc in range(n_suf_scat):
                    gpsimd.indirect_dma_start(
                        out=out_d[:],
                        out_offset=bass.IndirectOffsetOnAxis(
                            ap=idxs_t[:, cc : cc + 1], axis=0
                        ),
                        in_=sufx_t[:, cc * suf_w : (cc + 1) * suf_w],
                        in_offset=None,
                    ).then_inc(s_ss, 16)
            gpsimd.wait_ge(s_ss, 16 * n_suf_scat * n_iters)
            gpsimd.wait_ge(s_rs, 16 * n_suf_scat * n_iters)

    nc.compile()
    return nc


def _prepare(values, prefix_sum, suffix_mat):
    """Host-side sharding + index-table construction. Returns None if the
    fast path does not apply, else (build_args, in_maps, out_lens, n_out)."""
    B, SUF = suffix_mat.shape
    n_vals = values.shape[0]
    M = 8                      # cores
    SPC = B // M               # segments per core
    n_out = n_vals + B * SUF

    starts = np.concatenate([[0], prefix_sum[:-1]])
    lengths = prefix_sum - starts

    # fast path assumptions: segments tile values exactly, every segment
    # >= G (=> one boundary per grid block, repair fits in the segment),
    # and per-core row counts divide the 128 partitions evenly.
    G = 256
    if (
        int(prefix_sum[-1]) != n_vals
        or B % M != 0
        or SPC % 128 != 0
        or SUF != 256
        or lengths.min() < G
    ):
        return None

    # ---- per-core geometry (shapes identical across cores) ----
    core_base = np.array([starts[c * SPC] for c in range(M)] + [n_vals], np.int64)
    Ls = core_base[1:] - core_base[:-1]
    Lmax = int(Ls.max())

    W_FULL = 4096
    cols_tot = math.ceil(Lmax / 128 / G) * G          # cols, multiple of G
    k_full, rem = divmod(cols_tot, W_FULL)
    chunk_cols = [W_FULL] * k_full + ([rem] if rem else [])
    LPAD = 128 * cols_tot
    NBLK = LPAD // G
    n_idx_cols = cols_tot // G

    max_dummies = int(max(NBLK - (int(L) + G - 1) // G for L in Ls) + 2)
    SCR_SLOTS = max(2, max_dummies)
    SCR_BASE = LPAD + SPC * SUF
    out_len = SCR_BASE + SCR_SLOTS * G

    n_suf_scat = SPC // 128                           # suffix rows per partition
    suf_cols = n_suf_scat * SUF

    F = np.zeros(len(chunk_cols) + 1, np.int64)
    for g, w in enumerate(chunk_cols):
        F[g + 1] = F[g] + 128 * w

    in_maps = []
    out_lens = []
    for c in range(M):
        base = int(core_base[c])
        L = int(Ls[c])
        ends_local = (prefix_sum[c * SPC : (c + 1) * SPC] - base).astype(np.int64)

        # grid-block destinations
        sblk = np.arange(NBLK, dtype=np.int64) * G
        seg = np.searchsorted(ends_local, sblk, side="right")
        real = seg < SPC
        dst = sblk + G * seg
        dst[~real] = SCR_BASE + G * (np.nonzero(~real)[0] % SCR_SLOTS)
        dst = dst.astype(np.int32)

        idxv = np.empty((128, n_idx_cols), np.int32)
        colbase = 0
        for g, w in enumerate(chunk_cols):
            nb = 128 * w // G
            b0 = int(F[g]) // G
            idxv[:, colbase : colbase + w // G] = dst[b0 : b0 + nb].reshape(
                128, w // G
            )
            colbase += w // G

        vals_pad = np.zeros(LPAD, np.float32)
        vals_pad[:L] = values[base : base + L]

        # repair blocks: r=0 dummy -> scratch slot 1; r>=1 head of segment r
        rep_src = np.zeros(SPC, np.int64)
        rep_dst = np.full(SPC, SCR_BASE + G, np.int64)
        rep_src[1:] = ends_local[:-1]
        rep_dst[1:] = ends_local[:-1] + G * np.arange(1, SPC)
        rep_data = vals_pad[rep_src[:, None] + np.arange(G)[None, :]]
        idxr = rep_dst.astype(np.int32).reshape(128, n_suf_scat)

        # suffix destinations: row r -> end_of_values(seg r) + SUF*r
        dsts = (ends_local + SUF * np.arange(SPC, dtype=np.int64)).astype(np.int32)
        idxs = dsts.reshape(128, n_suf_scat)

        im = {
            "idxv": idxv,
            "idxs": idxs,
            "idxr": idxr,
            "sufx": np.ascontiguousarray(
                suffix_mat[c * SPC : (c + 1) * SPC].reshape(128, suf_cols)
            ),
            "repd": np.ascontiguousarray(rep_data.reshape(128, suf_cols)),
        }
        for g, w in enumerate(chunk_cols):
            im[f"vals{g}"] = vals_pad[F[g] : F[g + 1]].reshape(128, w)
        in_maps.append(im)
        out_lens.append(L + SPC * SUF)

    build_args = (chunk_cols, n_idx_cols, suf_cols, n_suf_scat, out_len, SUF)
    return build_args, in_maps, out_lens, n_out


def kernel(values, prefix_sum, suffix_mat):
    values = _np(values, np.float32)
    prefix_sum = _np(prefix_sum, np.int64)
    suffix_mat = _np(suffix_mat, np.float32)

    prep = _prepare(values, prefix_sum, suffix_mat)
    if prep is None:
        return _host_reference(values, prefix_sum, suffix_mat)
    build_args, in_maps, out_lens, n_out = prep

    nc = _build_program(*build_args)

    from concourse.bass_utils import run_bass_kernel_spmd

    res = run_bass_kernel_spmd(nc, in_maps, core_ids=list(range(8)))

    out = np.empty(n_out, np.float32)
    pos = 0
    for c in range(8):
        out[pos : pos + out_lens[c]] = res.results[c]["out"][: out_lens[c], 0]
        pos += out_lens[c]
    assert pos == n_out
    return out
